# revision 2
# baseline (speedup 1.0000x reference)
"""Trainium2 Bass kernel for the FGWF objective:

    out = sum(cost_mat(graph, graph_b, prob, prob_b, tran, embedding, embedding_b) * tran)

Decomposition (all heavy terms on device, final O(N) dots on host in f64):
  sum(cost*T) = sum_i (f1_i + 0.5*||E_i||^2) * rowsum(T)_i
              + sum_j (f2_j + 0.5*||Eb_j||^2) * colsum(T)_j
              - 2 * <G @ T, T @ Gb>              (two 2048^3 matmuls, bf16)
              - <E, T @ Eb>                      (feature cross term)
  with f1 = (G^2) @ p_s, f2 = (Gb^2) @ p_t.

Sharding over 8 NeuronCores: 2D grid (4 row-blocks x 2 col-blocks) for the
main matmuls; rank-1 terms sharded by rows/cols/embedding-half so every core
runs the identical SPMD program on different data.
"""
import numpy as np
import ml_dtypes

import concourse.bass as bass
from concourse import mybir
from concourse import bass_utils
from concourse.tile import TileContext

BF16 = ml_dtypes.bfloat16
F32 = mybir.dt.float32
BF = mybir.dt.bfloat16
N = 2048
NCORES = 8

_cache = {}


def _split_waits(nc):
    """The walrus build here rejects >1 sem-wait per instruction; hoist extra
    waits onto preceding same-engine nops."""
    MAXW = 1
    for fn in nc.m.functions:
        for b in fn.blocks:
            out = []
            changed = False
            for inst in b.instructions:
                si = inst.sync_info
                waits = list(si.on_wait) if si and si.on_wait else []
                if len(waits) > MAXW:
                    changed = True
                    k = 0
                    while len(waits) > MAXW:
                        chunk, waits = waits[:MAXW], waits[MAXW:]
                        out.append(mybir.InstNoOp(
                            name=f"{inst.name}-wsplit{k}", engine=inst.engine,
                            sync_info=mybir.SyncInfo(on_wait=chunk, on_update=[]),
                            bass_nofuse=True))
                        k += 1
                    inst.sync_info = mybir.SyncInfo(
                        on_wait=waits,
                        on_update=list(si.on_update) if si.on_update else [])
                out.append(inst)
            if changed:
                b.instructions = out


def _build():
    nc = bass.Bass()
    AL = mybir.AluOpType

    # per-core inputs (host-sharded)
    csT = nc.declare_dram_parameter("csT", [N, 512], BF, isOutput=False)
    tT = nc.declare_dram_parameter("tT", [N, 512], BF, isOutput=False)
    tcc = nc.declare_dram_parameter("tcc", [N, 1024], BF, isOutput=False)
    gbc = nc.declare_dram_parameter("gbc", [N, 1024], BF, isOutput=False)
    gbT = nc.declare_dram_parameter("gbT", [N, 256], BF, isOutput=False)
    ebf = nc.declare_dram_parameter("ebf", [N, 64], BF, isOutput=False)
    efT = nc.declare_dram_parameter("efT", [64, 512], F32, isOutput=False)
    ebs = nc.declare_dram_parameter("ebs", [256, 128], F32, isOutput=False)
    psb = nc.declare_dram_parameter("psb", [N], BF, isOutput=False)
    ptb = nc.declare_dram_parameter("ptb", [N], BF, isOutput=False)

    # outputs
    ocols_d = nc.declare_dram_parameter("out_cols", [128, 42], F32, isOutput=True)
    orows_d = nc.declare_dram_parameter("out_rows", [1, 1280], F32, isOutput=True)
    oemb_d = nc.declare_dram_parameter("out_emb", [64, 1], F32, isOutput=True)

    with TileContext(nc) as tc:
        with (
            tc.tile_pool(name="big", bufs=1) as big,
            tc.tile_pool(name="sqa", bufs=2) as sqa_p,
            tc.tile_pool(name="sqb", bufs=2) as sqb_p,
            tc.tile_pool(name="cpa", bufs=2) as cpa_p,
            tc.tile_pool(name="tout", bufs=2) as tout_p,
            tc.tile_pool(name="pa", bufs=3, space="PSUM") as pa_p,
            tc.tile_pool(name="pb", bufs=3, space="PSUM") as pb_p,
            tc.tile_pool(name="pe", bufs=1, space="PSUM") as pe_p,
            tc.tile_pool(name="prow", bufs=1, space="PSUM") as prow_p,
        ):
            csT_sb = big.tile([128, 16, 512], BF, tag="csT")
            tT_sb = big.tile([128, 16, 512], BF, tag="tT")
            tcc_sb = big.tile([128, 16, 1024], BF, tag="tcc")
            gbc_sb = big.tile([128, 16, 1024], BF, tag="gbc")
            gbT_sb = big.tile([128, 16, 256], BF, tag="gbT")
            ebf_sb = big.tile([128, 16, 64], BF, tag="ebf")
            efT_sb = big.tile([64, 512], F32, tag="efT")
            ebs_sb = big.tile([128, 2, 128], F32, tag="ebs")
            psb_sb = big.tile([128, 16], BF, tag="psb")
            ptb_sb = big.tile([128, 16], BF, tag="ptb")
            ones64 = big.tile([64, 1], F32, tag="ones")
            sqe_sb = big.tile([64, 512], F32, tag="sqe")
            toute_sb = big.tile([64, 512], F32, tag="toute")
            ocols = big.tile([128, 42], F32, tag="ocols")
            orows = big.tile([1, 1280], F32, tag="orows")
            oemb = big.tile([64, 1], F32, tag="oemb")

            csT_ap = csT.rearrange("(t p) w -> p t w", p=128)
            tT_ap = tT.rearrange("(t p) w -> p t w", p=128)
            tcc_ap = tcc.rearrange("(t p) w -> p t w", p=128)
            gbc_ap = gbc.rearrange("(t p) w -> p t w", p=128)
            gbT_ap = gbT.rearrange("(t p) w -> p t w", p=128)
            ebf_ap = ebf.rearrange("(t p) w -> p t w", p=128)

            # tiny params first (cheap, unblock f1/f2 early)
            nc.sync.dma_start(out=psb_sb[:], in_=psb.rearrange("(t p) -> p t", p=128))
            nc.sync.dma_start(out=ptb_sb[:], in_=ptb.rearrange("(t p) -> p t", p=128))
            # k-chunked loads of the four big operands: 4 ktiles per chunk
            for q in range(4):
                ks = slice(4 * q, 4 * q + 4)
                nc.sync.dma_start(out=csT_sb[:, ks, :], in_=csT_ap[:, ks, :])
                nc.sync.dma_start(out=tcc_sb[:, ks, :], in_=tcc_ap[:, ks, :])
                nc.sync.dma_start(out=tT_sb[:, ks, :], in_=tT_ap[:, ks, :])
                nc.sync.dma_start(out=gbc_sb[:, ks, :], in_=gbc_ap[:, ks, :])
            nc.sync.dma_start(out=gbT_sb[:], in_=gbT_ap[:])
            nc.sync.dma_start(out=ebf_sb[:], in_=ebf_ap[:])
            nc.sync.dma_start(out=efT_sb[:], in_=efT[:])
            nc.sync.dma_start(out=ebs_sb[:], in_=ebs.rearrange("(s p) d -> p s d", p=128))
            nc.vector.memset(ones64[:], 1.0)

            # ---- main: S_main partials over 8 (n, m) pairs ----
            pair = 0
            for n in range(2):
                nsl = slice(512 * n, 512 * n + 512)
                for m in range(4):
                    msl = slice(128 * m, 128 * m + 128)
                    pa = pa_p.tile([128, 512], F32)
                    pb = pb_p.tile([128, 512], F32)
                    for k in range(16):
                        nc.tensor.matmul(pa[:], csT_sb[:, k, msl], tcc_sb[:, k, nsl],
                                         start=(k == 0), stop=(k == 15))
                        nc.tensor.matmul(pb[:], tT_sb[:, k, msl], gbc_sb[:, k, nsl],
                                         start=(k == 0), stop=(k == 15))
                    ca = cpa_p.tile([128, 512], F32)
                    nc.vector.tensor_copy(ca[:], pa[:])
                    to = tout_p.tile([128, 512], F32)
                    nc.vector.scalar_tensor_tensor(
                        out=to[:], in0=ca[:], scalar=1.0, in1=pb[:],
                        op0=AL.mult, op1=AL.mult,
                        accum_out=ocols[:, pair:pair + 1])
                    pair += 1

            # ---- S_emb: psum_E = Eb_half^T @ T[rblk]^T = (T Eb)^T  [64, 512] ----
            pe_ = pe_p.tile([64, 512], F32)
            for j in range(16):
                nc.tensor.matmul(pe_[:], ebf_sb[:, j, :], tT_sb[:, j, :],
                                 start=(j == 0), stop=(j == 15))
            nc.vector.scalar_tensor_tensor(
                out=toute_sb[:], in0=efT_sb[:], scalar=1.0, in1=pe_[:],
                op0=AL.mult, op1=AL.mult, accum_out=oemb[:, 0:1])

            # ---- f1 row [1,512]: sum_k p_s[k] * G[i,k]^2 ----
            pf1 = prow_p.tile([1, 512], F32, tag="prow")
            for k in range(16):
                s = sqa_p.tile([128, 512], BF)
                nc.vector.tensor_mul(s[:], csT_sb[:, k, :], csT_sb[:, k, :])
                nc.tensor.matmul(pf1[:], psb_sb[:, k:k + 1], s[:],
                                 start=(k == 0), stop=(k == 15))
            nc.vector.tensor_copy(orows[:1, 0:512], pf1[:])

            # ---- f2 row [1,256]: sum_k p_t[k] * Gb[j,k]^2, j in jslice ----
            pf2 = prow_p.tile([1, 512], F32, tag="prow")
            for k in range(16):
                s = sqb_p.tile([128, 256], BF)
                nc.vector.tensor_mul(s[:], gbT_sb[:, k, :], gbT_sb[:, k, :])
                nc.tensor.matmul(pf2[:1, 0:256], ptb_sb[:, k:k + 1], s[:],
                                 start=(k == 0), stop=(k == 15))
            nc.vector.tensor_copy(orows[:1, 512:768], pf2[:1, 0:256])

            # ---- ||E_i||^2 (this d-half) row [1,512] ----
            nc.vector.tensor_mul(sqe_sb[:], efT_sb[:], efT_sb[:])
            pne = prow_p.tile([1, 512], F32, tag="prow")
            nc.tensor.matmul(pne[:], ones64[:], sqe_sb[:], start=True, stop=True)
            nc.vector.tensor_copy(orows[:1, 768:1280], pne[:])

            # ---- ||Eb_j||^2 for jslice -> ocols[:, 40:42] ----
            for s2 in range(2):
                to = tout_p.tile([128, 512], F32)
                nc.vector.scalar_tensor_tensor(
                    out=to[:, 0:128], in0=ebs_sb[:, s2, :], scalar=1.0,
                    in1=ebs_sb[:, s2, :], op0=AL.mult, op1=AL.mult,
                    accum_out=ocols[:, 40 + s2:41 + s2])

            # ---- rowsum(T) partial over ccols -> ocols[:, 8:24] ----
            for t in range(16):
                nc.vector.reduce_sum(ocols[:, 8 + t:9 + t], tcc_sb[:, t, :],
                                     axis=mybir.AxisListType.X)
            # ---- colsum(T) partial over rblk -> ocols[:, 24:40] ----
            for t in range(16):
                nc.vector.reduce_sum(ocols[:, 24 + t:25 + t], tT_sb[:, t, :],
                                     axis=mybir.AxisListType.X)

            nc.sync.dma_start(out=ocols_d[:], in_=ocols[:])
            nc.sync.dma_start(out=orows_d[:], in_=orows[:1, :])
            nc.sync.dma_start(out=oemb_d[:], in_=oemb[:])

    _split_waits(nc)
    return nc


def _prep_inputs(graph, embedding, prob, graph_b, embedding_b, prob_b, tran):
    bf = lambda x: np.ascontiguousarray(x).astype(BF16)
    G = np.asarray(graph, np.float32)
    E = np.asarray(embedding, np.float32)
    P = np.asarray(prob, np.float32).reshape(N)
    GB = np.asarray(graph_b, np.float32)
    EB = np.asarray(embedding_b, np.float32)
    PB = np.asarray(prob_b, np.float32).reshape(N)
    T = np.asarray(tran, np.float32)

    psb = bf(P)
    ptb = bf(PB)
    in_maps = []
    for idx in range(NCORES):
        r, c = idx // 2, idx % 2
        rblk = slice(512 * r, 512 * r + 512)
        ccols = slice(1024 * c, 1024 * c + 1024)
        dh = slice(64 * c, 64 * c + 64)
        jsl = slice(256 * idx, 256 * idx + 256)
        in_maps.append({
            "csT": bf(G[rblk, :].T),
            "tT": bf(T[rblk, :].T),
            "tcc": bf(T[:, ccols]),
            "gbc": bf(GB[:, ccols]),
            "gbT": bf(GB[jsl, :].T),
            "ebf": bf(EB[:, dh]),
            "efT": np.ascontiguousarray(E[rblk, dh].T, dtype=np.float32),
            "ebs": np.ascontiguousarray(EB[jsl, :], dtype=np.float32),
            "psb": psb,
            "ptb": ptb,
        })
    return in_maps


def _reduce(results):
    S_main = 0.0
    S_emb = 0.0
    f1 = np.zeros(N, np.float64)
    f2 = np.zeros(N, np.float64)
    r = np.zeros(N, np.float64)
    c = np.zeros(N, np.float64)
    nE = np.zeros(N, np.float64)
    nEB = np.zeros(N, np.float64)
    for idx in range(NCORES):
        rr, cc = idx // 2, idx % 2
        rblk = slice(512 * rr, 512 * rr + 512)
        jsl = slice(256 * idx, 256 * idx + 256)
        ocols = np.asarray(results[idx]["out_cols"], np.float64)
        orows = np.asarray(results[idx]["out_rows"], np.float64)[0]
        oemb = np.asarray(results[idx]["out_emb"], np.float64)
        S_main += ocols[:, 0:8].sum()
        S_emb += oemb.sum()
        f2[jsl] = orows[512:768]
        nEB[jsl] = ocols[:, 40:42].T.reshape(256)
        nE[rblk] += orows[768:1280]
        if cc == 0:
            f1[rblk] = orows[0:512]
            # colsum partial over rblk: tile-major [128,16] -> j = t*128+p
            c += ocols[:, 24:40].T.reshape(N)
        if rr == 0:
            r += ocols[:, 8:24].T.reshape(N)
    total = (
        ((f1 + 0.5 * nE) * r).sum()
        + ((f2 + 0.5 * nEB) * c).sum()
        - 2.0 * S_main
        - S_emb
    )
    return np.float32(total)


def run_spmd(in_maps, trace=False, **kw):
    if "nc" not in _cache:
        _cache["nc"] = _build()
    return bass_utils.run_bass_kernel_spmd(
        _cache["nc"], in_maps, list(range(NCORES)), trace=trace, **kw)


def kernel(graph, embedding, prob, graph_b, embedding_b, prob_b, tran,
           weights, ole_coeff, idx):
    in_maps = _prep_inputs(graph, embedding, prob, graph_b, embedding_b,
                           prob_b, tran)
    res = run_spmd(in_maps)
    return _reduce(res.results)
